# revision 3
# baseline (speedup 1.0000x reference)
"""MoE kernel for Trainium2 (8 NeuronCores, expert-parallel dense-masked).

Strategy:
- Expert parallelism: core e computes expert e's SwiGLU FFN over ALL tokens,
  with the top-2 gate folded into the mid activations (non-selected tokens get
  gate 0, contributing exactly 0). Router is computed redundantly on every
  core; each core's Wg is column-permuted so its own expert is column 0
  (keeps the SPMD program identical across cores).
- Shared expert: tensor-parallel over the hidden dim (176 of 1408 per core).
- Everything runs transposed ([feature, token] layout) so all matmuls contract
  over the partition dim with no on-device transposes of activations.
- Each core emits a partial y^T [D, N]; host sums the 8 partials (the output
  is sum-sharded) and transposes back.
"""

import numpy as np

import concourse.bacc as bacc
import concourse.mybir as mybir
import concourse.tile as tile
from concourse.bass_utils import run_bass_kernel_spmd
from concourse.masks import make_identity

# Problem shapes (hardcoded per contract).
B, T, D = 2, 2048, 1024
E, TOPK, H = 8, 2, 704
SH = 1408
N = B * T            # 4096 tokens
HP = 768             # H padded to a multiple of 128
SHS = SH // E        # 176 shared-hidden cols per core
SHSP = 256           # padded for clean [128, 2, ...] SBUF layout
NT = 8               # token chunks
TOK = N // NT        # 512
KD = D // 128        # 8
HC = HP // 128       # 6

F32 = mybir.dt.float32
F32R = mybir.dt.float32r

_cache = {}


def _build_nc():
    nc = bacc.Bacc("TRN2", target_bir_lowering=False, debug=False, num_devices=8)

    xt = nc.dram_tensor("xt", [D, N], F32, kind="ExternalInput")
    w13 = nc.dram_tensor("w13", [D, 2 * HP], F32, kind="ExternalInput")
    w2 = nc.dram_tensor("w2", [HP, D], F32, kind="ExternalInput")
    wsh = nc.dram_tensor("wsh", [D, 2 * SHS], F32, kind="ExternalInput")
    ws2 = nc.dram_tensor("ws2", [SHSP, D], F32, kind="ExternalInput")
    wg = nc.dram_tensor("wg", [D, E], F32, kind="ExternalInput")
    yt = nc.dram_tensor("yt", [D, N], F32, kind="ExternalOutput")

    with tile.TileContext(nc) as tc:
        with (
            tc.tile_pool(name="wpool", bufs=1) as wpool,
            tc.tile_pool(name="xpool", bufs=2) as xpool,
            tc.tile_pool(name="apool", bufs=14) as apool,
            tc.tile_pool(name="aspool", bufs=4) as aspool,
            tc.tile_pool(name="opool", bufs=4) as opool,
            tc.tile_pool(name="gpool", bufs=3) as gpool,
            tc.tile_pool(name="ps_hg", bufs=4, space="PSUM") as ps_hg,
            tc.tile_pool(name="ps_y", bufs=2, space="PSUM") as ps_y,
            tc.tile_pool(name="ps_g", bufs=2, space="PSUM") as ps_g,
        ):
            # Constants
            id_sb = wpool.tile([128, 128], F32, tag="ident")
            make_identity(nc, id_sb[:])
            ones_f32 = wpool.tile([1, 128], F32, tag="ones32")
            nc.vector.memset(ones_f32[:], 1.0)
            ones_sb = wpool.tile([1, 128], F32R, tag="ones")
            nc.vector.tensor_copy(ones_sb[:], ones_f32[:])

            # Resident weights (loaded once).
            w13_sb = wpool.tile([128, KD, 2 * HP], F32R, tag="w13")
            nc.sync.dma_start(
                w13_sb[:], w13.ap().bitcast(F32R).rearrange("(k p) m -> p k m", p=128)
            )
            w2_sb = wpool.tile([128, HC, D], F32R, tag="w2")
            nc.sync.dma_start(
                w2_sb[:], w2.ap().bitcast(F32R).rearrange("(k p) m -> p k m", p=128)
            )
            wsh_sb = wpool.tile([128, KD, 2 * SHS], F32R, tag="wsh")
            nc.sync.dma_start(
                wsh_sb[:], wsh.ap().bitcast(F32R).rearrange("(k p) m -> p k m", p=128)
            )
            ws2_sb = wpool.tile([128, 2, D], F32R, tag="ws2")
            nc.sync.dma_start(
                ws2_sb[:], ws2.ap().bitcast(F32R).rearrange("(k p) m -> p k m", p=128)
            )
            # Router weights stay true fp32: top-2 selection must match the
            # fp32 reference bit-for-bit on near-tie tokens, and f32r's ~1e-4
            # noise flips them. The fp32 4-pass matmul is cheap at M=8.
            wg_sb = wpool.tile([128, KD, E], F32, tag="wg")
            nc.sync.dma_start(
                wg_sb[:], wg.ap().rearrange("(k p) m -> p k m", p=128)
            )

            xt_r = xt.ap().bitcast(F32R).rearrange("(k p) n -> p k n", p=128)

            for t in range(NT):
                ts = slice(t * TOK, (t + 1) * TOK)
                x_sb = xpool.tile([128, KD, TOK], F32R, tag="x")
                nc.sync.dma_start(x_sb[:], xt_r[:, :, ts])

                # --- Router: logits [E, TOK] ---
                ps_l = ps_g.tile([E, TOK], F32, tag="gm")
                for kk in range(KD):
                    nc.tensor.matmul(
                        ps_l[:], wg_sb[:, kk, :], x_sb[:, kk, :].bitcast(F32),
                        start=(kk == 0), stop=(kk == KD - 1),
                    )
                logit_sb = gpool.tile([E, TOK], F32, tag="logit")
                nc.vector.tensor_copy(logit_sb[:], ps_l[:])

                # --- Gate math in token-major layout ---
                # Transpose logits quarters: [E, 128] -> [128, E], batched in
                # one PSUM tile [128, 4*E].
                ps_q = ps_g.tile([128, 4 * E], F32, tag="gm")
                for q in range(4):
                    nc.tensor.transpose(
                        ps_q[:, q * E:(q + 1) * E],
                        logit_sb[:, q * 128:(q + 1) * 128],
                        id_sb[:E, :E],
                    )
                # e = exp(logits) (softmax normalization cancels out of the
                # final gate ratio, so it is skipped).
                e_sb = gpool.tile([128, 4 * E], F32, tag="e")
                nc.scalar.activation(e_sb[:], ps_q[:], mybir.ActivationFunctionType.Exp)
                e3 = e_sb[:].rearrange("p (q k) -> p q k", k=E)
                v1 = gpool.tile([128, 4], F32, tag="v1")
                nc.vector.reduce_max(v1[:], e3, axis=mybir.AxisListType.X)
                v2 = gpool.tile([128, 4], F32, tag="v2")
                for q in range(4):
                    eq = gpool.tile([128, E], F32, tag="eq")
                    nc.vector.tensor_scalar(
                        eq[:], e_sb[:, q * E:(q + 1) * E], v1[:, q:q + 1], None,
                        op0=mybir.AluOpType.is_equal,
                    )
                    nc.vector.tensor_mul(eq[:], eq[:], e_sb[:, q * E:(q + 1) * E])
                    nc.vector.tensor_sub(eq[:], e_sb[:, q * E:(q + 1) * E], eq[:])
                    nc.vector.reduce_max(
                        v2[:, q:q + 1], eq[:], axis=mybir.AxisListType.X
                    )
                den = gpool.tile([128, 4], F32, tag="den")
                nc.vector.tensor_add(den[:], v1[:], v2[:])
                rden = gpool.tile([128, 4], F32, tag="rden")
                nc.vector.reciprocal(rden[:], den[:])
                # Own expert is always column 0 (host permutes Wg per core).
                e0 = gpool.tile([128, 4], F32, tag="e0")
                nc.vector.tensor_copy(e0[:], e3[:, :, 0])
                sel = gpool.tile([128, 4], F32, tag="sel")
                nc.vector.tensor_tensor(
                    sel[:], e0[:], v2[:], op=mybir.AluOpType.is_ge
                )
                gate = gpool.tile([128, 4], F32, tag="gate")
                nc.vector.tensor_mul(gate[:], e0[:], sel[:])
                nc.vector.tensor_mul(gate[:], gate[:], rden[:])
                # Transpose back to a [1, TOK] row, then broadcast to all
                # 128 partitions with a K=1 matmul against ones.
                ps_t2 = ps_g.tile([1, TOK], F32, tag="gm")
                for q in range(4):
                    nc.tensor.transpose(
                        ps_t2[:, q * 128:(q + 1) * 128], gate[:, q:q + 1], id_sb[:]
                    )
                grow_sb = gpool.tile([1, TOK], F32R, tag="grow")
                nc.vector.tensor_copy(grow_sb[:], ps_t2[:])
                ps_gb = ps_g.tile([128, TOK], F32, tag="gm")
                nc.tensor.matmul(
                    ps_gb[:], ones_sb[:], grow_sb[:], start=True, stop=True
                )
                gb_sb = gpool.tile([128, TOK], F32R, tag="gb")
                nc.vector.tensor_copy(gb_sb[:], ps_gb[:])

                # --- Shared expert up-proj (2 col-chunks: 128 + 48) ---
                as_list = []
                for (c1, c2, w) in ((0, SHS, 128), (128, SHS + 128, SHS - 128)):
                    ph = ps_hg.tile([128, TOK], F32, tag="hg")
                    for kk in range(KD):
                        nc.tensor.matmul(
                            ph[:w], wsh_sb[:, kk, c1:c1 + w], x_sb[:, kk, :],
                            start=(kk == 0), stop=(kk == KD - 1),
                        )
                    pg = ps_hg.tile([128, TOK], F32, tag="hg")
                    for kk in range(KD):
                        nc.tensor.matmul(
                            pg[:w], wsh_sb[:, kk, c2:c2 + w], x_sb[:, kk, :],
                            start=(kk == 0), stop=(kk == KD - 1),
                        )
                    a_sh = aspool.tile([128, TOK], F32R, tag="ash")
                    nc.scalar.activation(
                        a_sh[:w], ph[:w], mybir.ActivationFunctionType.Silu
                    )
                    nc.vector.tensor_mul(a_sh[:w], a_sh[:w], pg[:w])
                    as_list.append((a_sh, w))

                # --- Expert up-proj (6 col-chunks of 128, H padded to 768) ---
                a_list = []
                for hc in range(HC):
                    ph = ps_hg.tile([128, TOK], F32, tag="hg")
                    for kk in range(KD):
                        nc.tensor.matmul(
                            ph[:], w13_sb[:, kk, hc * 128:(hc + 1) * 128],
                            x_sb[:, kk, :],
                            start=(kk == 0), stop=(kk == KD - 1),
                        )
                    pg = ps_hg.tile([128, TOK], F32, tag="hg")
                    for kk in range(KD):
                        nc.tensor.matmul(
                            pg[:], w13_sb[:, kk, HP + hc * 128:HP + (hc + 1) * 128],
                            x_sb[:, kk, :],
                            start=(kk == 0), stop=(kk == KD - 1),
                        )
                    a_sb = apool.tile([128, TOK], F32R, tag="a")
                    nc.scalar.activation(
                        a_sb[:], ph[:], mybir.ActivationFunctionType.Silu
                    )
                    nc.vector.tensor_mul(a_sb[:], a_sb[:], pg[:])
                    nc.vector.tensor_mul(a_sb[:], a_sb[:], gb_sb[:])
                    a_list.append(a_sb)

                # --- Down-proj: expert (6 K-chunks) + shared (2 K-chunks)
                # accumulated into one PSUM bank per D-chunk ---
                for dc in range(KD):
                    cs = slice(dc * 128, (dc + 1) * 128)
                    py = ps_y.tile([128, TOK], F32, tag="y")
                    for kc in range(HC):
                        nc.tensor.matmul(
                            py[:], w2_sb[:, kc, cs], a_list[kc][:],
                            start=(kc == 0), stop=False,
                        )
                    for kc in range(2):
                        a_sh, w = as_list[kc]
                        nc.tensor.matmul(
                            py[:], ws2_sb[:w, kc, cs], a_sh[:w],
                            start=False, stop=(kc == 1),
                        )
                    o_sb = opool.tile([128, TOK], F32, tag="o")
                    nc.vector.tensor_copy(o_sb[:], py[:])
                    nc.sync.dma_start(yt.ap()[cs, ts], o_sb[:])

    nc.compile()
    return nc


def _prep_inputs(x, Wg, W1, W3, W2, Ws1, Ws3, Ws2):
    xf = np.ascontiguousarray(x.reshape(N, D).T)  # [D, N]
    in_maps = []
    for e in range(E):
        w13 = np.zeros((D, 2 * HP), np.float32)
        w13[:, :H] = W1[e]
        w13[:, HP:HP + H] = W3[e]
        w2 = np.zeros((HP, D), np.float32)
        w2[:H] = W2[e]
        s = SHS * e
        wsh = np.concatenate([Ws1[:, s:s + SHS], Ws3[:, s:s + SHS]], axis=1)
        ws2 = np.zeros((SHSP, D), np.float32)
        ws2[:SHS] = Ws2[s:s + SHS]
        perm = [e] + [i for i in range(E) if i != e]
        wg = np.ascontiguousarray(Wg[perm].T)  # [D, E], own expert first
        in_maps.append({
            "xt": xf,
            "w13": w13,
            "w2": np.ascontiguousarray(w2),
            "wsh": np.ascontiguousarray(wsh),
            "ws2": ws2,
            "wg": wg,
        })
    return in_maps


def kernel(**inputs):
    if "nc" not in _cache:
        _cache["nc"] = _build_nc()
    nc = _cache["nc"]
    in_maps = _prep_inputs(
        inputs["x"], inputs["Wg"], inputs["W1"], inputs["W3"], inputs["W2"],
        inputs["Ws1"], inputs["Ws3"], inputs["Ws2"],
    )
    res = run_bass_kernel_spmd(nc, in_maps, core_ids=list(range(8)))
    acc = res.results[0]["yt"].astype(np.float32)
    for c in range(1, 8):
        acc = acc + res.results[c]["yt"]
    return np.ascontiguousarray(acc.T).reshape(B, T, D)


# revision 7
# speedup vs baseline: 1.0999x; 1.0999x over previous
"""MoE kernel for Trainium2 (8 NeuronCores, expert-parallel dense-masked).

Strategy:
- Expert parallelism: core e computes expert e's SwiGLU FFN over ALL tokens,
  with the top-2 gate folded into the mid activations (non-selected tokens get
  gate 0, contributing exactly 0). The router is computed redundantly on every
  core; each core's Wg is column-permuted so its own expert is column 0
  (keeps the SPMD program identical across cores).
- Router exactness: top-2 selection must match the fp32 reference on near-tie
  tokens, but the PE's f32r path truncates operands to ~fp22 (~1e-4 noise,
  enough to flip ties). The router is therefore computed in split precision:
  x and Wg are decomposed on the host into hi (13-bit mantissa) + lo parts,
  and logits = xh@Wgh + xh@Wgl + xlo@Wgh accumulate in one PSUM group. hi
  parts pass through the fp22 truncation unchanged, so the result is
  fp32-accurate (~1e-7) at 3 single-pass matmuls instead of a 4-pass fp32 one.
  The FFN consumes xh directly - identical to what the PE's truncation of the
  full x would produce.
- Shared expert: data-parallel over tokens. Tokens are rotated per core on
  the host so each core's own 512-token slice lands in the LAST chunk; the
  full-width (1408) shared FFN runs only on that chunk with zero
  partial-chunk waste, and its weights stream in during earlier chunks.
- Everything runs transposed ([feature, token] layout) so all matmuls
  contract over the partition dim with no on-device transposes of
  activations.
- Each core emits a partial y^T [D, N]; the host un-rotates, sums the 8
  partials (the output is sum-sharded) and transposes back.
"""

import numpy as np

import concourse.bacc as bacc
import concourse.mybir as mybir
import concourse.tile as tile
from concourse.bass_utils import run_bass_kernel_spmd
from concourse.masks import make_identity

# Problem shapes (hardcoded per contract).
B, T, D = 2, 2048, 1024
E, TOPK, H = 8, 2, 704
SH = 1408
N = B * T            # 4096 tokens
HP = 768             # H padded to a multiple of 128
NT = 8               # token chunks
TOK = N // NT        # 512
KD = D // 128        # 8
HC = HP // 128       # 6
SHC = SH // 128      # 11
SHARED_T = NT - 1    # chunk that carries this core's shared-FFN tokens

F32 = mybir.dt.float32
F32R = mybir.dt.float32r

_cache = {}


def _build_nc():
    nc = bacc.Bacc("TRN2", target_bir_lowering=False, debug=False, num_devices=8)

    xt = nc.dram_tensor("xt", [D, N], F32, kind="ExternalInput")
    xlo = nc.dram_tensor("xlo", [D, N], F32, kind="ExternalInput")
    w13 = nc.dram_tensor("w13", [D, 2 * HP], F32, kind="ExternalInput")
    w2 = nc.dram_tensor("w2", [HP, D], F32, kind="ExternalInput")
    wsf = nc.dram_tensor("wsf", [D, 2 * SH], F32, kind="ExternalInput")
    ws2f = nc.dram_tensor("ws2f", [SH, D], F32, kind="ExternalInput")
    wg = nc.dram_tensor("wg", [D, 2 * E], F32, kind="ExternalInput")
    yt = nc.dram_tensor("yt", [D, N], F32, kind="ExternalOutput")

    with tile.TileContext(nc) as tc:
        with (
            tc.tile_pool(name="wpool", bufs=1) as wpool,
            tc.tile_pool(name="swupool", bufs=3) as swupool,
            tc.tile_pool(name="swdpool", bufs=11) as swdpool,
            tc.tile_pool(name="xpool", bufs=3) as xpool,
            tc.tile_pool(name="xlopool", bufs=1) as xlopool,
            tc.tile_pool(name="apool", bufs=7) as apool,
            tc.tile_pool(name="asfpool", bufs=11) as asfpool,
            tc.tile_pool(name="opool", bufs=2) as opool,
            tc.tile_pool(name="gpool", bufs=2) as gpool,
            tc.tile_pool(name="ps_hg", bufs=4, space="PSUM") as ps_hg,
            tc.tile_pool(name="ps_y", bufs=2, space="PSUM") as ps_y,
            tc.tile_pool(name="ps_g", bufs=2, space="PSUM") as ps_g,
        ):
            # Constants
            id_sb = wpool.tile([128, 128], F32, tag="ident")
            make_identity(nc, id_sb[:])
            ones_f32 = wpool.tile([1, 128], F32, tag="ones32")
            nc.vector.memset(ones_f32[:], 1.0)
            ones_sb = wpool.tile([1, 128], F32R, tag="ones")
            nc.vector.tensor_copy(ones_sb[:], ones_f32[:])

            xt_r = xt.ap().bitcast(F32R).rearrange("(k p) n -> p k n", p=128)
            xlo_r = xlo.ap().bitcast(F32R).rearrange("(k p) n -> p k n", p=128)
            w13_r = w13.ap().bitcast(F32R).rearrange("(k p) m -> p k m", p=128)
            w2_r = w2.ap().bitcast(F32R).rearrange("(k p) m -> p k m", p=128)
            wsf_r = wsf.ap().bitcast(F32R).rearrange("(k p) m -> p k m", p=128)
            ws2f_r = ws2f.ap().bitcast(F32R).rearrange("(k p) m -> p k m", p=128)

            # Router weights [wgh | wgl], own expert in column 0 of each half.
            wg_sb = wpool.tile([128, KD, 2 * E], F32R, tag="wg")
            nc.sync.dma_start(
                wg_sb[:], wg.ap().bitcast(F32R).rearrange("(k p) m -> p k m", p=128)
            )

            # Expert weights, resident; loaded in per-chunk slices so the
            # first token chunk's matmuls aren't gated on one huge DMA.
            w13_sb = wpool.tile([128, KD, 2 * HP], F32R, tag="w13")
            for mc in range(2 * HC):
                nc.sync.dma_start(
                    w13_sb[:, :, mc * 128:(mc + 1) * 128],
                    w13_r[:, :, mc * 128:(mc + 1) * 128],
                )
            w2_sb = wpool.tile([128, HC, D], F32R, tag="w2")
            for kc in range(HC):
                nc.sync.dma_start(w2_sb[:, kc:kc + 1, :], w2_r[:, kc:kc + 1, :])

            for t in range(NT):
                ts = slice(t * TOK, (t + 1) * TOK)
                xh0 = xpool.tile([128, KD // 2, TOK], F32R, tag="x")
                nc.sync.dma_start(xh0[:], xt_r[:, 0:KD // 2, ts])
                xh1 = xpool.tile([128, KD // 2, TOK], F32R, tag="x")
                nc.sync.dma_start(xh1[:], xt_r[:, KD // 2:KD, ts])
                xk = lambda kk: (xh0 if kk < KD // 2 else xh1)[:, kk % (KD // 2), :]
                xlo_sb = xlopool.tile([128, KD, TOK], F32R, tag="xlo")
                nc.sync.dma_start(xlo_sb[:], xlo_r[:, :, ts])

                # --- Router: logits [E, TOK], split-precision f32r ---
                ps_l = ps_g.tile([E, TOK], F32, tag="gm")
                n_mm = 3 * KD
                i = 0
                for kk in range(KD):
                    for (wcol, xin) in (
                        (0, xk(kk)), (E, xk(kk)), (0, xlo_sb[:, kk, :])
                    ):
                        nc.tensor.matmul(
                            ps_l[:], wg_sb[:, kk, wcol:wcol + E], xin,
                            start=(i == 0), stop=(i == n_mm - 1),
                        )
                        i += 1
                logit_sb = gpool.tile([E, TOK], F32, tag="logit")
                nc.vector.tensor_copy(logit_sb[:], ps_l[:])

                # --- Gate math in token-major layout ---
                ps_q = ps_g.tile([128, 4 * E], F32, tag="gm")
                for q in range(4):
                    nc.tensor.transpose(
                        ps_q[:, q * E:(q + 1) * E],
                        logit_sb[:, q * 128:(q + 1) * 128],
                        id_sb[:E, :E],
                    )
                # e = exp(logits); softmax normalization cancels in the gate.
                e_sb = gpool.tile([128, 4 * E], F32, tag="e")
                nc.scalar.activation(e_sb[:], ps_q[:], mybir.ActivationFunctionType.Exp)
                e3 = e_sb[:].rearrange("p (q k) -> p q k", k=E)
                v1 = gpool.tile([128, 4], F32, tag="v1")
                nc.vector.reduce_max(v1[:], e3, axis=mybir.AxisListType.X)
                v2 = gpool.tile([128, 4], F32, tag="v2")
                for q in range(4):
                    eq = gpool.tile([128, E], F32, tag="eq")
                    nc.vector.tensor_scalar(
                        eq[:], e_sb[:, q * E:(q + 1) * E], v1[:, q:q + 1], None,
                        op0=mybir.AluOpType.is_equal,
                    )
                    nc.vector.tensor_mul(eq[:], eq[:], e_sb[:, q * E:(q + 1) * E])
                    nc.vector.tensor_sub(eq[:], e_sb[:, q * E:(q + 1) * E], eq[:])
                    nc.vector.reduce_max(
                        v2[:, q:q + 1], eq[:], axis=mybir.AxisListType.X
                    )
                den = gpool.tile([128, 4], F32, tag="den")
                nc.vector.tensor_add(den[:], v1[:], v2[:])
                rden = gpool.tile([128, 4], F32, tag="rden")
                nc.vector.reciprocal(rden[:], den[:])
                # Own expert is always column 0 (host permutes Wg per core).
                e0 = gpool.tile([128, 4], F32, tag="e0")
                nc.vector.tensor_copy(e0[:], e3[:, :, 0])
                sel = gpool.tile([128, 4], F32, tag="sel")
                nc.vector.tensor_tensor(
                    sel[:], e0[:], v2[:], op=mybir.AluOpType.is_ge
                )
                gate = gpool.tile([128, 4], F32, tag="gate")
                nc.vector.tensor_mul(gate[:], e0[:], sel[:])
                nc.vector.tensor_mul(gate[:], gate[:], rden[:])
                # Transpose back to a [1, TOK] row, broadcast via K=1 matmul.
                ps_t2 = ps_g.tile([1, TOK], F32, tag="gm")
                for q in range(4):
                    nc.tensor.transpose(
                        ps_t2[:, q * 128:(q + 1) * 128], gate[:, q:q + 1], id_sb[:]
                    )
                grow_sb = gpool.tile([1, TOK], F32R, tag="grow")
                nc.vector.tensor_copy(grow_sb[:], ps_t2[:])
                ps_gb = ps_g.tile([128, TOK], F32, tag="gm")
                nc.tensor.matmul(
                    ps_gb[:], ones_sb[:], grow_sb[:], start=True, stop=True
                )
                gb_sb = gpool.tile([128, TOK], F32R, tag="gb")
                nc.vector.tensor_copy(gb_sb[:], ps_gb[:])

                # --- Expert up-proj (6 col-chunks of 128, H padded to 768) ---
                a_list = []
                for hc in range(HC):
                    ph = ps_hg.tile([128, TOK], F32, tag="hg")
                    for kk in range(KD):
                        nc.tensor.matmul(
                            ph[:], w13_sb[:, kk, hc * 128:(hc + 1) * 128],
                            xk(kk),
                            start=(kk == 0), stop=(kk == KD - 1),
                        )
                    pg = ps_hg.tile([128, TOK], F32, tag="hg")
                    for kk in range(KD):
                        nc.tensor.matmul(
                            pg[:], w13_sb[:, kk, HP + hc * 128:HP + (hc + 1) * 128],
                            xk(kk),
                            start=(kk == 0), stop=(kk == KD - 1),
                        )
                    a_sb = apool.tile([128, TOK], F32R, tag="a")
                    nc.scalar.activation(
                        a_sb[:], ph[:], mybir.ActivationFunctionType.Silu
                    )
                    nc.vector.tensor_mul(a_sb[:], a_sb[:], pg[:])
                    nc.vector.tensor_mul(a_sb[:], a_sb[:], gb_sb[:])
                    a_list.append(a_sb)

                # --- Shared expert, full width, only on this core's own
                # token chunk (host rotates tokens so it's chunk NT-1) ---
                as_full = []
                if t == SHARED_T:
                    for sc in range(SHC):
                        ph = ps_hg.tile([128, TOK], F32, tag="hg")
                        for kk in range(KD):
                            nc.tensor.matmul(
                                ph[:], _sw(nc, swupool, wsf_r, sc)[:, kk, :],
                                xk(kk),
                                start=(kk == 0), stop=(kk == KD - 1),
                            )
                        pg = ps_hg.tile([128, TOK], F32, tag="hg")
                        for kk in range(KD):
                            nc.tensor.matmul(
                                pg[:], _sw(nc, swupool, wsf_r, SHC + sc)[:, kk, :],
                                xk(kk),
                                start=(kk == 0), stop=(kk == KD - 1),
                            )
                        a_sh = asfpool.tile([128, TOK], F32R, tag="asf")
                        nc.scalar.activation(
                            a_sh[:], ph[:], mybir.ActivationFunctionType.Silu
                        )
                        nc.vector.tensor_mul(a_sh[:], a_sh[:], pg[:])
                        as_full.append(a_sh)

                # --- Down-proj: expert (+ shared on the last chunk) ---
                for dc in range(KD):
                    cs = slice(dc * 128, (dc + 1) * 128)
                    py = ps_y.tile([128, TOK], F32, tag="y")
                    n_k = HC + (SHC if t == SHARED_T else 0)
                    ki = 0
                    for kc in range(HC):
                        nc.tensor.matmul(
                            py[:], w2_sb[:, kc, cs], a_list[kc][:],
                            start=(ki == 0), stop=(ki == n_k - 1),
                        )
                        ki += 1
                    if t == SHARED_T:
                        for sc in range(SHC):
                            nc.tensor.matmul(
                                py[:], _sw2(nc, swdpool, ws2f_r, sc, dc),
                                as_full[sc][:],
                                start=(ki == 0), stop=(ki == n_k - 1),
                            )
                            ki += 1
                    o_sb = opool.tile([128, TOK], F32, tag="o")
                    nc.vector.tensor_copy(o_sb[:], py[:])
                    nc.sync.dma_start(yt.ap()[cs, ts], o_sb[:])

    nc.compile()
    return nc


_sw_cache = {}


def _sw(nc, swupool, wsf_r, mc):
    """Stream one [128, KD, 128] up-proj column block of the shared weights."""
    key = ("up", mc)
    if key not in _sw_cache:
        t = swupool.tile([128, KD, 128], F32R, tag="swu")
        nc.sync.dma_start(t[:], wsf_r[:, :, mc * 128:(mc + 1) * 128])
        _sw_cache[key] = t
    return _sw_cache[key]


def _sw2(nc, swdpool, ws2f_r, sc, dc):
    """Stream one [128, D/2] down-proj K-block half of the shared weights.

    Keyed by (sc, half) so each half-D pass of the dc loop holds 11 blocks of
    [128, 512] instead of [128, 1024] - halves the resident footprint at the
    cost of re-reading Ws2 once.
    """
    half = dc // 4
    key = ("dn", sc, half)
    if key not in _sw_cache:
        t = swdpool.tile([128, 1, D // 2], F32R, tag="swd")
        nc.sync.dma_start(
            t[:], ws2f_r[:, sc:sc + 1, half * 512:(half + 1) * 512]
        )
        _sw_cache[key] = t
    return _sw_cache[key][:, 0, (dc % 4) * 128:(dc % 4 + 1) * 128]


def _m13(a):
    """Truncate fp32 mantissa to 13 bits (survives the PE's fp22 read)."""
    return (a.view(np.uint32) & np.uint32(0xFFFFFC00)).view(np.float32)


def _prep_inputs(x, Wg, W1, W3, W2, Ws1, Ws3, Ws2):
    xf = np.ascontiguousarray(x.reshape(N, D).T).astype(np.float32)  # [D, N]
    xh = _m13(xf)
    xlo = xf - xh
    wsf = np.concatenate([Ws1, Ws3], axis=1)  # [D, 2*SH]
    in_maps = []
    for e in range(E):
        sh = (SHARED_T - e) % NT * TOK  # roll tokens: own slice -> chunk NT-1
        w13 = np.zeros((D, 2 * HP), np.float32)
        w13[:, :H] = W1[e]
        w13[:, HP:HP + H] = W3[e]
        w2 = np.zeros((HP, D), np.float32)
        w2[:H] = W2[e]
        perm = [e] + [i for i in range(E) if i != e]
        wgp = Wg[perm].T.astype(np.float32)  # [D, E], own expert first
        wgh = _m13(wgp)
        wgl = wgp - wgh
        in_maps.append({
            "xt": np.roll(xh, sh, axis=1),
            "xlo": np.roll(xlo, sh, axis=1),
            "w13": w13,
            "w2": np.ascontiguousarray(w2),
            "wsf": np.ascontiguousarray(wsf),
            "ws2f": np.ascontiguousarray(Ws2),
            "wg": np.ascontiguousarray(np.concatenate([wgh, wgl], axis=1)),
        })
    return in_maps


def kernel(**inputs):
    if "nc" not in _cache:
        _sw_cache.clear()
        _cache["nc"] = _build_nc()
    nc = _cache["nc"]
    in_maps = _prep_inputs(
        inputs["x"], inputs["Wg"], inputs["W1"], inputs["W3"], inputs["W2"],
        inputs["Ws1"], inputs["Ws3"], inputs["Ws2"],
    )
    res = run_bass_kernel_spmd(nc, in_maps, core_ids=list(range(8)))
    acc = None
    for e in range(8):
        sh = (SHARED_T - e) % NT * TOK
        part = np.roll(res.results[e]["yt"], -sh, axis=1)
        acc = part if acc is None else acc + part
    return np.ascontiguousarray(acc.T).reshape(B, T, D)
